# revision 70
# baseline (speedup 1.0000x reference)
"""Trainium2 Bass kernel for nn_MultiAttention (GQA+MLA attention, gated, SwiGLU out).

Sharding (8 cores, core c = b*4 + g):
- Attention: data-parallel over batch b, tensor-parallel over KV head g
  (4 q-heads + 1 kv head per core), all 2048 queries, causal structure
  identical on every core (single SPMD program).
- Reshard: two 8-rank AllToAlls, round m carries local head pair {2m, 2m+1}
  in bf16 (each source duplicates its blocks for both batches' consumers;
  destinations select their batch's half with 0/1 mask inputs). Round 0
  fires halfway through attention and overlaps the remaining pair's compute.
- MLP: data-parallel over batch, sharded over token quarter (t-slice of 512),
  full DFF per core, bf16 weights/activations with fp32 PSUM accumulation.
  Output per core is y[b, 512g:512(g+1), :] transposed.

Attention matmuls run in float32r (tf32-like, full PE rate at N>=512).
Rope is matmul-based: a second projection against host-half-swapped weights
produces the rotated tensor (no SBUF shuffle DMAs); K row duplication is
folded into host-duplicated Wk.
Softmax: no max-subtraction (scores bounded), denominator via an appended
ones-column in V (psum row 64), normalization via K=1 broadcast matmul.
Tails are split: psO extraction (DVE copy+recip, frees the PSUM bank fast)
runs inline; the gate/normalize math is deferred two chunks so the PE never
waits on it. Gate matmul uses raw psO (column scaling commutes).
Phase A transcendentals use only the Exp table (sigmoid via exp trick);
phase B uses the Silu table directly from PSUM (one table switch).
MLP2 accumulates all 32 k-tiles in a single PSUM bank (no vector adds).
DMAs are batched (packed tables, grouped x/readback transfers) to minimize
per-DMA descriptor-generation overhead.
"""

import numpy as np
import ml_dtypes

import concourse.bacc as bacc
import concourse.bass as bass
import concourse.mybir as mybir
import concourse.tile as tile
from concourse.bass_utils import run_bass_kernel_spmd

# problem dims
B, T, DM = 2, 2048, 1024
H, HKV, DH = 16, 4, 64
LAT, DFF = 64, 4096
SCALE = DH ** -0.5
ROPE_BASE = 10000.0

N_CORES = 8
TCH = 512                 # t-chunk (matmul moving dim)
NCH = T // TCH            # 4 chunks
NHL = H // HKV            # 4 local q-heads per core
QT = T // N_CORES * 2     # 512 = MLP token slice per core
f32 = mybir.dt.float32
f32r = mybir.dt.float32r
bf16 = mybir.dt.bfloat16

_cache = {}
import os
DEBUG_TAP = os.environ.get("KTAP", "")  # qpre/kpre/oraw/ot16/attf
PHASES = "all"  # "A" / "B" for sim attribution
SKIP_ATT = False   # sim: skip attention loop
SKIP_TAIL = False  # sim: skip normalize/gate tail
X_BUFS = 2
P_BUFS = 6
PSS_BUFS = 2
PSO_BUFS = 2
TMP_BUFS = 2
W1_BUFS = 2
PSH_BUFS = 6
PSY_BUFS = 2
TAIL_DEFER = 1


def _build_nc():
    nc = bacc.Bacc("TRN2", target_bir_lowering=False, debug=False,
                   num_devices=N_CORES)

    # ---- DRAM I/O ----
    xT = nc.dram_tensor("xT", [DM, T], f32r, kind="ExternalInput")
    wq = nc.dram_tensor("wq", [DM, NHL * DH], f32r, kind="ExternalInput")
    wqr = nc.dram_tensor("wqr", [DM, NHL * DH], f32r, kind="ExternalInput")
    wkv = nc.dram_tensor("wkv", [DM, LAT], f32r, kind="ExternalInput")
    # wkk = wkd | wkdr | wv | wg packed along columns: [64, 128+128+64+64]
    wkk = nc.dram_tensor("wkk", [LAT, 384], f32r, kind="ExternalInput")
    bgn = nc.dram_tensor("bgn", [128, 1], f32, kind="ExternalInput")  # NEGATED bias, x2
    wo1 = nc.dram_tensor("wo1", [DM, DFF], bf16, kind="ExternalInput")
    wo2 = nc.dram_tensor("wo2", [DFF, DM], bf16, kind="ExternalInput")
    # tabs = cos | nsin | head-duplicated dmask packed: [128, 2T + 8*TCH]
    tabs = nc.dram_tensor("tabs", [128, 2 * T + 8 * TCH], f32,
                          kind="ExternalInput")
    selAB = nc.dram_tensor("selAB", [128, 2], f32, kind="ExternalInput")
    yT = nc.dram_tensor("yT", [DM, QT], f32, kind="ExternalOutput")

    with tile.TileContext(nc) as tc:
        _body(nc, tc, xT, wq, wqr, wkv, wkk, bgn, wo1, wo2, tabs, selAB, yT)
    nc.compile()
    return nc


def _body(nc, tc, xT, wq, wqr, wkv, wkk, bgn, wo1, wo2, tabs, selAB, yT):
    Exp = mybir.ActivationFunctionType.Exp
    Silu = mybir.ActivationFunctionType.Silu
    KT_DM = DM // 128  # 8 k-tiles over model dim
    KT_FF = DFF // 128  # 32 k-tiles over ff dim

    with (
        tc.tile_pool(name="const", bufs=1) as const,
        tc.tile_pool(name="tmp", bufs=TMP_BUFS) as tmp,
        tc.tile_pool(name="dram", bufs=1, space="DRAM") as dram,
    ):
        # reshard buffers: round m carries local head pair {2m, 2m+1}
        cin = [dram.tile([8, 2 * DH, QT], bf16, name=f"cin{r}")
               for r in range(2)]
        cout = [dram.tile([8, 2 * DH, QT], bf16, name=f"cout{r}")
                for r in range(2)]

        # ---- constants / small weights (live whole kernel) ----
        tabs_t = const.tile([128, 2 * T + 8 * TCH], f32r)
        cs_t = tabs_t[:, 0:T]
        ns_t = tabs_t[:, T:2 * T]
        dmask_t = tabs_t[:, 2 * T:2 * T + 8 * TCH]  # [4 s-tiles][2 heads][TCH]
        wkk_t = const.tile([LAT, 384], f32r)
        wkd_t = wkk_t[:, 0:128]
        wkdr_t = wkk_t[:, 128:256]
        wv_t = wkk_t[:, 256:320]
        wg_t = wkk_t[:, 320:384]
        bgn_t = const.tile([128, 1], f32)
        selAB_t = const.tile([128, 2], f32)
        ones16 = const.tile([128, 16], f32)
        nc.any.memset(ones16[:], 1.0)

        def _const_dmas():
            nc.sync.dma_start(out=wkk_t[:], in_=wkk[:, :])
            nc.sync.dma_start(out=tabs_t[:, 0:2 * T],
                              in_=tabs[:, 0:2 * T].bitcast(f32r))
            nc.sync.dma_start(out=tabs_t[:, 2 * T:],
                              in_=tabs[:, 2 * T:].bitcast(f32r))
            nc.sync.dma_start(out=bgn_t[:], in_=bgn[:, :])
            nc.sync.dma_start(out=selAB_t[:], in_=selAB[:, :])

        # ================= Phase A: projections + rope + attention ==========
        if PHASES in ("all", "A"):
          with (
            tc.tile_pool(name="xw", bufs=1) as xw,
            tc.tile_pool(name="qk", bufs=1) as qk,
            tc.tile_pool(name="ppool", bufs=3) as ppool,
            tc.tile_pool(name="psA", bufs=1, space="PSUM") as psum,
        ):
            wq_t = xw.tile([128, KT_DM, NHL * DH], f32r)
            wqr_t = xw.tile([128, KT_DM, NHL * DH], f32r)
            wkv_t = xw.tile([128, KT_DM, LAT], f32r)
            dmask16 = qk.tile([128, 2, 128], bf16, name="dmask16")
            wq_r = wq[:].rearrange("(kt p) m -> p kt m", p=128)
            wqr_r = wqr[:].rearrange("(kt p) m -> p kt m", p=128)

            # projections + rope, CHUNK-GRANULAR tiles: separate tiles
            # per t-chunk so Tile's per-tile dep tracking lets attention on
            # chunk 0 start while later chunks are still projecting.
            q_pre = [[qk.tile([128, TCH], f32r, tag=f"q{m}", bufs=NCH,
                              name=f"q_pre{m}_{jc}") for jc in range(NCH)]
                     for m in range(2)]
            k_pre = [qk.tile([128, TCH], f32r, tag="kp", bufs=NCH,
                             name=f"k_pre{jc}") for jc in range(NCH)]
            lat_c = [qk.tile([LAT, TCH], f32r, tag="lat", bufs=NCH,
                             name=f"lat{jc}") for jc in range(NCH)]
            v_sb = [qk.tile([128, 4, 65], bf16, tag="vsb", bufs=NCH,
                            name=f"v_sb{jc}") for jc in range(NCH)]
            xT_r = xT[:].rearrange("(kt p) t -> p kt t", p=128)

            def rope_ps(out, ps, jc):
                # ps[:,0,:] holds the plain projection, ps[:,1,:] the
                # half-swapped one (from host-permuted weights):
                # out = ps0*cos + ps1*(+-sin), all straight from PSUM.
                cs = cs_t[:, jc * TCH:(jc + 1) * TCH]
                ns = ns_t[:, jc * TCH:(jc + 1) * TCH]
                sw = tmp.tile([128, TCH], f32r, tag="swap", bufs=2,
                              name="swap")
                nc.vector.tensor_mul(sw[:], ps[:, 1, :], ns)
                nc.vector.tensor_mul(out[:], ps[:, 0, :], cs)
                nc.vector.tensor_add(out[:], out[:], sw[:])

            for jc in range(NCH):
                # x tiles in two 4-kt groups (one DMA each; chunk 0 is
                # loaded k-tile-wise so the first matmul starts early)
                x_j = [xw.tile([128, 4, TCH], f32r, tag="x",
                               bufs=X_BUFS * 2, name=f"x_j{g}")
                       for g in range(2)]
                if jc == 0:
                    nc.sync.dma_start(out=x_j[0][:, 0, :],
                                      in_=xT_r[:, 0, 0:TCH])
                    nc.sync.dma_start(out=wq_t[:, 0, :], in_=wq_r[:, 0, :])
                    nc.sync.dma_start(
                        out=wq_t[:, 1:KT_DM, :],
                        in_=wq_r[:, 1:KT_DM, :])
                    for kt in range(1, KT_DM):
                        nc.sync.dma_start(out=x_j[kt // 4][:, kt % 4, :],
                                          in_=xT_r[:, kt, 0:TCH])
                    nc.sync.dma_start(out=wqr_t[:, 0, :],
                                      in_=wqr_r[:, 0, :])
                    nc.sync.dma_start(out=wqr_t[:, 1:KT_DM, :],
                                      in_=wqr_r[:, 1:KT_DM, :])
                    nc.sync.dma_start(
                        out=wkv_t[:],
                        in_=wkv[:].rearrange("(kt p) m -> p kt m", p=128))
                    _const_dmas()
                    nc.vector.tensor_copy(
                        dmask16[:],
                        dmask_t[:, 0:2 * TCH]
                        .rearrange("p (a t) -> p a t", a=2)[:, :, 0:128])
                else:
                    for g in range(2):
                        nc.sync.dma_start(
                            out=x_j[g][:],
                            in_=xT_r[:, 4 * g:4 * g + 4,
                                     jc * TCH:(jc + 1) * TCH])

                def xop(kt):
                    return x_j[kt // 4][:, kt % 4, :]

                for m in range(2):
                    ps = psum.tile([128, 2, TCH], f32, tag="psS",
                                   bufs=PSS_BUFS, name="psq")
                    for kt in range(KT_DM):
                        nc.tensor.matmul(
                            ps[:, 0, :], wq_t[:, kt, m * 128:(m + 1) * 128],
                            xop(kt),
                            start=(kt == 0), stop=(kt == KT_DM - 1),
                        )
                    for kt in range(KT_DM):
                        nc.tensor.matmul(
                            ps[:, 1, :], wqr_t[:, kt, m * 128:(m + 1) * 128],
                            xop(kt),
                            start=(kt == 0), stop=(kt == KT_DM - 1),
                        )
                    if jc == 0 and m == 0 and DEBUG_TAP == "psq":
                        d0 = tmp.tile([128, TCH], f32, tag="swap", name="d0")
                        nc.vector.tensor_copy(d0[:], ps[:, 0, :])
                        nc.sync.dma_start(out=yT[0:128, :], in_=d0[:])
                        d1 = tmp.tile([128, TCH], f32, tag="swap", name="d1")
                        nc.vector.tensor_copy(d1[:], ps[:, 1, :])
                        nc.sync.dma_start(out=yT[128:256, :], in_=d1[:])
                    rope_ps(q_pre[m][jc], ps, jc)
                ps = psum.tile([64, TCH], f32, tag="psS", bufs=PSS_BUFS,
                               name="pslat")
                for kt in range(KT_DM):
                    nc.tensor.matmul(ps[:], wkv_t[:, kt, :], xop(kt),
                                     start=(kt == 0), stop=(kt == KT_DM - 1))
                nc.vector.tensor_copy(lat_c[jc][:], ps[:])
                # k chunk: one matmul against host-duplicated wk produces
                # both 64-row copies; rot variant likewise.
                ps = psum.tile([128, 2, TCH], f32, tag="psS", bufs=PSS_BUFS,
                               name="psk")
                nc.tensor.matmul(ps[:, 0, :], wkd_t[:], lat_c[jc][:],
                                 start=True, stop=True)
                nc.tensor.matmul(ps[:, 1, :], wkdr_t[:], lat_c[jc][:],
                                 start=True, stop=True)
                rope_ps(k_pre[jc], ps, jc)
                if jc == 0 and DEBUG_TAP == "qpre":
                    nc.sync.dma_start(out=yT[0:128, :],
                                      in_=q_pre[0][0][:].bitcast(f32))
                if jc == 0 and DEBUG_TAP == "kpre":
                    nc.sync.dma_start(out=yT[0:128, :],
                                      in_=k_pre[0][:].bitcast(f32))
                # v chunk (+ones column at index 64)
                nc.vector.tensor_copy(
                    v_sb[jc][:, :, 64:65].rearrange("p a x -> p (a x)"),
                    ones16[:, 0:4])
                for tl in range(4):
                    ps = psum.tile([128, 64], f32, tag="psS", bufs=PSS_BUFS,
                                   name="psv")
                    nc.tensor.matmul(
                        ps[:], lat_c[jc][:, tl * 128:(tl + 1) * 128],
                        wv_t[:], start=True, stop=True)
                    nc.vector.tensor_copy(v_sb[jc][:, tl, 0:64], ps[:])

            # attention, one HEAD PAIR at a time: head a = 2m (q/k rows
            # 0-63), head b = 2m+1 (rows 64-127). QK matmuls of the two
            # heads are adjacent K=64 row-group tiles and run concurrently
            # on the PE array; gate/denominator matmuls pair via col-groups.
            # psO extraction runs inline (frees the bank fast); the
            # gate/normalize tail is deferred TAIL_DEFER chunks so the PE
            # never waits on its DVE chain.
            def tail2(m, j, oraw_a, oraw_b, rden_a, rden_b, oT16a, oT16b):
                # psDG shares the psO tag: psO/psDG allocations alternate,
                # so the 2-buf rotation double-buffers both (8-bank budget).
                # Gate matmuls on RAW psO (column scaling commutes); the
                # 1/denom column factors are partition-broadcast to base-0
                # tiles on gpsimd so every op stays lane-aligned and every
                # matmul output starts at partition 0 (walrus requirement).
                psDG = psum.tile([64, 2, TCH], f32, tag="psO",
                                 bufs=PSO_BUFS, name="psDG")
                nc.tensor.matmul(psDG[:, 0, :], wg_t[:], oraw_a[:],
                                 start=True, stop=True)
                nc.tensor.matmul(psDG[:, 1, :], wg_t[:], oraw_b[:],
                                 start=True, stop=True)
                dfacA = tmp.tile([64, TCH], f32, tag="dfac", bufs=2,
                                 name="dfacA")
                dfacB = tmp.tile([64, TCH], f32, tag="dfac", bufs=2,
                                 name="dfacB")
                # partition_broadcast only reads partition 0: bridge the
                # denominator rows down with two tiny lane-shift DMAs.
                rd0 = tmp.tile([1, 2, TCH], f32, tag="rd0", bufs=1,
                               name="rd0")
                nc.sync.dma_start(out=rd0[0:1, 0, :], in_=rden_a[64:65, :])
                nc.sync.dma_start(out=rd0[0:1, 1, :], in_=rden_b[64:65, :])
                nc.gpsimd.partition_broadcast(dfacA[:], rd0[0:1, 0, :])
                nc.gpsimd.partition_broadcast(dfacB[:], rd0[0:1, 1, :])
                garg = tmp.tile([64, 2, TCH], f32, tag="garg", bufs=1,
                                name="garg")
                nc.vector.tensor_mul(garg[:, 0, :], psDG[:, 0, :], dfacA[:])
                nc.vector.tensor_mul(garg[:, 1, :], psDG[:, 1, :], dfacB[:])
                eg = tmp.tile([64, 2, TCH], f32, tag="eg", name="eg")
                egf = eg[:].rearrange("p a t -> p (a t)")
                nc.scalar.activation(egf, garg[:].rearrange("p a t -> p (a t)"),
                                     Exp, bias=bgn_t[0:64, :], scale=-1.0)
                nc.vector.tensor_scalar_add(egf, egf, 1.0)
                nc.vector.reciprocal(egf, egf)
                nc.vector.tensor_mul(eg[:, 0, :], eg[:, 0, :], dfacA[:])
                nc.vector.tensor_mul(eg[:, 1, :], eg[:, 1, :], dfacB[:])
                js = slice(j * TCH, (j + 1) * TCH)
                nc.vector.tensor_mul(oT16a[0:64, js], oraw_a[:],
                                     eg[:, 0, :])
                nc.vector.tensor_mul(oT16b[0:64, js], oraw_b[:],
                                     eg[:, 1, :])

            pendq = []

            def flush_tail2(n):
                while len(pendq) > n:
                    tail2(*pendq.pop(0))

            for m in range(2 if not SKIP_ATT else 0):
                oT16a = qk.tile([64, T], bf16, tag="oT16", bufs=2,
                                name=f"oT16a_{m}")
                oT16b = qk.tile([64, T], bf16, tag="oT16", bufs=2,
                                name=f"oT16b_{m}")
                for j in range(NCH):
                    psO = psum.tile([65, 2, TCH], f32, tag="psO",
                                    bufs=PSO_BUFS, name="psO")
                    n_s = 4 * (j + 1)
                    for st in range(n_s):
                        # diagonal tile i: queries < 128*i can't see these
                        # keys — restrict QK/PV to columns >= lo and mask
                        # only the 128-col micro-block (same triangle for
                        # every i).
                        lo = max(0, st - 4 * j) * 128
                        psS = psum.tile([128, 2, TCH], f32, tag="psS",
                                        bufs=PSS_BUFS, name="psS")
                        for hh in range(2):
                            nc.tensor.matmul(
                                psS[:, hh, lo:TCH],
                                k_pre[st // 4][64 * hh:64 * hh + 64,
                                               (st % 4) * 128:
                                               (st % 4 + 1) * 128],
                                q_pre[m][j][64 * hh:64 * hh + 64, lo:TCH],
                                start=True, stop=True,
                            )
                        pt = ppool.tile([128, 2, TCH], bf16, tag="P",
                                        bufs=P_BUFS, name="P")
                        nc.scalar.activation(
                            pt[:, :, lo:TCH], psS[:, :, lo:TCH],
                            Exp, scale=SCALE,
                        )
                        if st >= 4 * j:  # diagonal micro-block mask
                            # pair 1's masks run on DVE so the Pool queue is
                            # free for the round-0 collective trigger.
                            meng = nc.gpsimd if m == 0 else nc.vector
                            meng.tensor_mul(
                                pt[:, :, lo:lo + 128],
                                pt[:, :, lo:lo + 128],
                                dmask16[:],
                            )
                        for hh in range(2):
                            nc.tensor.matmul(
                                psO[:, hh, lo:TCH],
                                v_sb[st // 4][:, st % 4, :],
                                pt[:, hh, lo:TCH],
                                start=(st == 0), stop=(st == n_s - 1),
                            )
                    if not SKIP_TAIL:
                        # deferred tail first so the DVE FIFO has ready work
                        # while this chunk's last PV matmuls finish
                        flush_tail2(TAIL_DEFER - 1)
                        # inline psO extraction: lane-aligned copies/recips
                        oraw_a = tmp.tile([64, TCH], f32r, tag="oraw",
                                          bufs=4, name="oraw_a")
                        oraw_b = tmp.tile([64, TCH], f32r, tag="oraw",
                                          bufs=4, name="oraw_b")
                        nc.vector.tensor_copy(oraw_a[:], psO[0:64, 0, :])
                        nc.vector.tensor_copy(oraw_b[:], psO[0:64, 1, :])
                        rden_a = tmp.tile([65, TCH], f32, tag="rden",
                                          bufs=3, name="rden_a")
                        rden_b = tmp.tile([65, TCH], f32, tag="rden",
                                          bufs=3, name="rden_b")
                        nc.vector.reciprocal(rden_a[64:65, :],
                                             psO[64:65, 0, :])
                        nc.vector.reciprocal(rden_b[64:65, :],
                                             psO[64:65, 1, :])
                        if m == 0 and j == 0 and DEBUG_TAP == "oraw":
                            nc.sync.dma_start(out=yT[0:64, :],
                                              in_=oraw_a[:].bitcast(f32))
                            nc.sync.dma_start(out=yT[64:129, :],
                                              in_=rden_a[:].bitcast(f32))
                        pendq.append((m, j, oraw_a, oraw_b, rden_a, rden_b,
                                      oT16a, oT16b))
                if not SKIP_TAIL:
                    flush_tail2(0)
                    if m == 0 and DEBUG_TAP == "ot16":
                        t32 = qk.tile([64, TCH], f32, tag="dbg", bufs=2,
                                      name="dbg")
                        nc.vector.tensor_copy(t32[:], oT16a[0:64, 0:TCH])
                        nc.sync.dma_start(out=yT[0:64, :], in_=t32[:])
                        t32b = qk.tile([64, TCH], f32, tag="dbg", bufs=2,
                                       name="dbgb")
                        nc.vector.tensor_copy(t32b[:], oT16b[0:64, 0:TCH])
                        nc.sync.dma_start(out=yT[64:128, :], in_=t32b[:])
                    for half in range(2):
                        # SBUF-side APs must stay partition-major
                        nc.sync.dma_start(
                            out=cin[m][4 * half:4 * half + 4, 0:DH, :]
                            .rearrange("s p t -> p s t"),
                            in_=oT16a[:].rearrange("p (s t) -> p s t", s=4),
                        )
                        nc.sync.dma_start(
                            out=cin[m][4 * half:4 * half + 4, DH:2 * DH, :]
                            .rearrange("s p t -> p s t"),
                            in_=oT16b[:].rearrange("p (s t) -> p s t", s=4),
                        )
                    nc.gpsimd.collective_compute(
                        "AllToAll", mybir.AluOpType.bypass,
                        replica_groups=[list(range(8))],
                        ins=[cin[m][:].opt()], outs=[cout[m][:].opt()],
                    )
                    if m == 0 and DEBUG_TAP in ("cinv", "coutv"):
                        src = cin if DEBUG_TAP == "cinv" else cout
                        db16 = qk.tile([128, TCH], bf16, tag="dbg16",
                                       bufs=2, name="dbg16")
                        nc.sync.dma_start(out=db16[:], in_=src[0][0])
                        db32 = qk.tile([128, TCH], f32, tag="dbg32",
                                       bufs=2, name="dbg32")
                        nc.vector.tensor_copy(db32[:], db16[:])
                        nc.sync.dma_start(out=yT[0:128, :], in_=db32[:])

        # ================= Phase B: MLP =================
        if PHASES in ("all", "B"):
          with (
            tc.tile_pool(name="mlp", bufs=1) as mlp,
            tc.tile_pool(name="wstream", bufs=2) as wstream,
            tc.tile_pool(name="psB", bufs=1, space="PSUM") as psum,
        ):
            # readback + batch-select on gpsimd (idle in phase B; keeps DVE
            # free and its FIFO unpolluted). One grouped DMA per round/half;
            # bA/bB share the "oraw" tmp tag so the rotation WAR keeps the
            # scheduler from hoisting these ahead of phase-A tail work.
            attF = [mlp.tile([128, 4, QT], bf16, tag="attF", bufs=2,
                             name=f"attF{r}") for r in range(2)]
            for r in range(2):
                seng = nc.gpsimd if r == 0 else nc.vector
                bA = tmp.tile([128, 4, QT], bf16, tag="oraw", bufs=4,
                              name="bA")
                bB = tmp.tile([128, 4, QT], bf16, tag="oraw", bufs=4,
                              name="bB")
                nc.scalar.dma_start(
                    out=bA[:],
                    in_=cout[r][0:4].rearrange("s p t -> p s t"))
                nc.scalar.dma_start(
                    out=bB[:],
                    in_=cout[r][4:8].rearrange("s p t -> p s t"))
                seng.tensor_scalar_mul(
                    bA[:].rearrange("p a t -> p (a t)"),
                    bA[:].rearrange("p a t -> p (a t)"), selAB_t[:, 0:1])
                seng.tensor_scalar_mul(
                    bB[:].rearrange("p a t -> p (a t)"),
                    bB[:].rearrange("p a t -> p (a t)"), selAB_t[:, 1:2])
                seng.tensor_add(
                    attF[r][:].rearrange("p a t -> p (a t)"),
                    bA[:].rearrange("p a t -> p (a t)"),
                    bB[:].rearrange("p a t -> p (a t)"))

            def attFop(kt):
                return attF[kt % 2][:, kt // 2, :]

            # wo2 resident in SBUF (bf16, 64KB/partition), streamed in per-fg
            # chunks inside the MLP1 loop so no single DMA hogs an engine.
            wo2_sb = mlp.tile([128, KT_FF, DM], bf16, name="wo2_sb")
            wo2_ap = wo2[:].rearrange("(kt p) m -> p kt m", p=128)

            yT_r = yT[:].rearrange("(kt p) t -> p kt t", p=128)
            wo1_ap = wo1[:].rearrange("(kt p) f -> p kt f", p=128)

            # MLP1: psH accumulates all 8 k-tiles (evens first so partial
            # accumulation can overlap the round-1 collective), then one
            # Silu from PSUM straight to bf16 SBUF.
            hg = [mlp.tile([128, QT], bf16, tag="hT", bufs=KT_FF,
                           name=f"hg{i}") for i in range(KT_FF)]
            for fg in range(DFF // TCH):  # 8 groups of 512 ff dims
                w1 = wstream.tile([128, KT_DM, TCH], bf16, tag="w1",
                                  bufs=W1_BUFS, name="w1")
                nc.scalar.dma_start(
                    out=w1[:], in_=wo1_ap[:, :, fg * TCH:(fg + 1) * TCH])
                nc.scalar.dma_start(
                    out=wo2_sb[:, fg * 4:(fg + 1) * 4, :],
                    in_=wo2_ap[:, fg * 4:(fg + 1) * 4, :])
                for mt in range(TCH // 128):
                    psH = psum.tile([128, QT], f32, tag="psH", bufs=PSH_BUFS,
                                    name="psH")
                    for kt in [0, 2, 4, 6, 1, 3, 5, 7]:
                        nc.tensor.matmul(
                            psH[:], w1[:, kt, mt * 128:(mt + 1) * 128],
                            attFop(kt),
                            start=(kt == 0), stop=(kt == 7),
                        )
                    nc.scalar.activation(hg[fg * 4 + mt][:], psH[:], Silu)

            # MLP2: one PSUM bank accumulates the full DFF contraction
            # (32 matmuls), then a single copy out per dm-tile.
            for dmt in range(KT_DM):
                psY = psum.tile([128, QT], f32, tag="psY", bufs=PSY_BUFS,
                                name="psY")
                for i in range(KT_FF):
                    nc.tensor.matmul(
                        psY[:], wo2_sb[:, i, dmt * 128:(dmt + 1) * 128],
                        hg[i][:],
                        start=(i == 0), stop=(i == KT_FF - 1),
                    )
                y_sb = tmp.tile([128, QT], f32, tag="ysb", bufs=2,
                                name="y_sb")
                nc.vector.tensor_copy(y_sb[:], psY[:])
                nc.scalar.dma_start(out=yT_r[:, dmt, :], in_=y_sb[:])


def _host_prep(x, Wq, Wkv_down, Wk_up, Wv_up, Wgate, bgate, Wo1, Wo2):
    half = DH // 2
    pos = np.arange(T, dtype=np.float32)
    inv_freq = 1.0 / (ROPE_BASE ** (np.arange(half, dtype=np.float32) / half))
    ang = pos[:, None] * inv_freq            # [T, 32]
    cos_tab = np.cos(ang).T.astype(np.float32)   # [32, T]
    sin_tab = np.sin(ang).T.astype(np.float32)
    cs64 = np.concatenate([cos_tab, cos_tab], 0)          # [64, T]
    ns64 = np.concatenate([-sin_tab, sin_tab], 0)         # [64, T]
    cs2 = np.concatenate([cs64, cs64], 0)                 # [128, T]
    ns2 = np.concatenate([ns64, ns64], 0)

    # diagonal quad mask, duplicated per head half: s-tile i valid iff
    # c >= 128*i + r; layout [128, 4 s-tiles, 2 heads, TCH]
    r = np.arange(128)[:, None, None]
    i = np.arange(4)[None, :, None]
    c = np.arange(TCH)[None, None, :]
    dmask = (c >= 128 * i + r).astype(np.float32)       # [128, 4, TCH]
    dmask2 = np.stack([dmask, dmask], axis=2).reshape(128, 8 * TCH)
    tabs = np.ascontiguousarray(np.concatenate([cs2, ns2, dmask2], axis=1))

    wo1_bf = np.ascontiguousarray(Wo1.astype(ml_dtypes.bfloat16))
    wo2_bf = np.ascontiguousarray(Wo2.astype(ml_dtypes.bfloat16))

    def _halfswap(w):
        # swap the two 32-col halves of each 64-col head block
        c = w.shape[-1]
        return np.ascontiguousarray(
            w.reshape(w.shape[0], c // DH, 2, DH // 2)[:, :, ::-1, :]
            .reshape(w.shape[0], c))

    in_maps = []
    for core in range(N_CORES):
        b, g = divmod(core, 4)
        sa = 1.0 - b
        wq_c = Wq[:, g * NHL * DH:(g + 1) * NHL * DH]
        wk_c = Wk_up[:, g * DH:(g + 1) * DH]
        wk_dup = np.concatenate([wk_c, wk_c], axis=1)
        wkk = np.concatenate(
            [wk_dup, _halfswap(wk_dup),
             Wv_up[:, g * DH:(g + 1) * DH], Wgate], axis=1)
        selAB = np.zeros((128, 2), np.float32)
        selAB[:, 0] = sa
        selAB[:, 1] = 1.0 - sa
        in_maps.append({
            "xT": np.ascontiguousarray(x[b].T),
            "wq": np.ascontiguousarray(wq_c),
            "wqr": _halfswap(wq_c),
            "wkv": np.ascontiguousarray(Wkv_down),
            "wkk": np.ascontiguousarray(wkk),
            "bgn": np.ascontiguousarray(
                np.concatenate([-bgate, -bgate])[:, None]),
            "wo1": wo1_bf,
            "wo2": wo2_bf,
            "tabs": tabs,
            "selAB": selAB,
        })
    return in_maps


def kernel(**inputs) -> np.ndarray:
    if "nc" not in _cache:
        _cache["nc"] = _build_nc()
    nc = _cache["nc"]
    in_maps = _host_prep(**inputs)
    res = run_bass_kernel_spmd(nc, in_maps, core_ids=list(range(N_CORES)))
    y = np.empty((B, T, DM), np.float32)
    for core in range(N_CORES):
        b, g = divmod(core, 4)
        y[b, g * QT:(g + 1) * QT, :] = res.results[core]["yT"].T
    return y


if __name__ == "__main__":
    import reference
    inputs = {k: np.asarray(v) for k, v in reference.setup_inputs().items()}
    out = kernel(**inputs)
    want = np.asarray(reference.reference(**inputs))
    err = np.abs(out - want).max()
    rel = err / np.abs(want).max()
    print(f"max abs err {err:.4e}, rel {rel:.4e}")
